# revision 17
# baseline (speedup 1.0000x reference)
"""QLoRA-style MLP (fake-quant base + fp32 LoRA + exact GeLU) on 8 TRN2 cores.

Sharding: token data-parallel (4096 tokens / 8 cores = 512 tokens per core),
weights replicated.  The only cross-core communication is a tiny AllReduce(max)
for the global fake-quant scale of the hidden activation.

Math per layer (matching the jax reference):
    base = fq(x) @ fq(W) + b          fq(t) = clip(round(t/s), -127, 127) * s,
                                      s = max(max|t|, 1e-8) / 127  (global max)
    lora = 2.0 * (x @ A) @ B          (low-rank path, bf16 operands on device)
    out  = base + lora                (layer 1 additionally GeLU'd)

Device mapping (per core, T=512 tokens):
  L1: psum[ff128, T] = sum_k qW_fc[k,ff]^T-tiles @ qxT[k,T]   (bf16 exact ints)
                       + B_fc[16,ff]^T @ ((x@A_fc)^T * 2/s1)  (bf16, K=16)
      hT = Gelu(psum * s1 + b_fc); abs-max tracked; per-mi epilogue also
      accumulates xa2 = A_proj^T @ h (bf16) into a persistent psum bank and
      spills h (fp32) to HBM for mi >= HKEEP (first HKEEP tiles stay in SBUF).
  barrier: local max -> PE-transpose/DVE-reduce to a scalar replicated on all
      128 partitions (exact) -> AllReduce(max) over 8 cores -> scales.
      Meanwhile (s_h-independent): lora2b[tok,d] = xa2^T @ (2 B_proj) + b_proj
      is computed into SBUF, and h reloads stream in on the gpsimd queue.
  L2: qhT = round(hT/s_h) as bf16 (magic-number rounding)
      psum[tok128, d] = sum_k qhT[k,tok]-tiles @ qW_proj[k,d]
      out = psum * s2 + lora2b
"""

import os
import sys

import numpy as np

if "/opt/trn_rl_repo" not in sys.path:
    sys.path.insert(0, "/opt/trn_rl_repo")

import ml_dtypes

# Problem shapes (hardcoded per contract).
B_, S, D, DFF, R = 2, 2048, 2048, 8192, 16
T = B_ * S  # 4096 tokens
NCORES = 8
TC = T // NCORES  # 512 tokens per core
QMAX = np.float32(127.0)
MAGIC = float(np.float32(12582912.0))  # 1.5 * 2**23: fp32 round-half-even trick

KO1 = D // 128  # 16  k-tiles for layer 1
MO1 = DFF // 512  # 16  512-wide ff blocks
M64 = DFF // 128  # 64  128-wide ff blocks
KO2 = DFF // 128  # 64  k-tiles for layer 2
NO2 = D // 512  # 4   512-wide output-col blocks
MT = TC // 128  # 4   token tiles per core
HKEEP = 20  # leading h tiles kept resident in SBUF (not spilled)

_CACHE = {}
LAST_RESULT = None  # test harness can read exec_time_ns etc. from here


def _build_nc(n_cores=NCORES, tc_=TC, d_=D, dff_=DFF, dmodel_=D, act="gelu", flags=()):
    """Build + compile the Bass program. Dimensions parameterizable for sim tests."""
    from contextlib import ExitStack

    import concourse.bass as bass  # noqa: F401
    import concourse.mybir as mybir
    import concourse.tile as tile
    from concourse import bacc
    from concourse.bass import ds, ts

    f32 = mybir.dt.float32
    bf16 = mybir.dt.bfloat16
    AF = mybir.ActivationFunctionType
    ALU = mybir.AluOpType

    ko1 = d_ // 128
    mo1 = dff_ // 512
    m64 = dff_ // 128
    ko2 = dff_ // 128
    no2 = dmodel_ // 512
    mt = tc_ // 128
    hkeep = min(HKEEP, m64)

    nc = bacc.Bacc(None, target_bir_lowering=False, num_devices=n_cores)

    # ---- kernel I/O -------------------------------------------------------
    qx_t = nc.dram_tensor("qx_t", [128, ko1, tc_], bf16, kind="ExternalInput")
    xt_t = nc.dram_tensor("xt_t", [128, ko1, tc_], bf16, kind="ExternalInput")
    wfc_t = nc.dram_tensor("wfc_t", [mo1, ko1, 128, 512], bf16, kind="ExternalInput")
    afc_t = nc.dram_tensor("afc_t", [128, ko1, R], bf16, kind="ExternalInput")
    bfcl_t = nc.dram_tensor("bfcl_t", [R, dff_], bf16, kind="ExternalInput")
    biasfc_t = nc.dram_tensor("biasfc_t", [128, m64], f32, kind="ExternalInput")
    wproj_t = nc.dram_tensor("wproj_t", [ko2, 128, no2, 512], bf16, kind="ExternalInput")
    aproj_t = nc.dram_tensor("aproj_t", [128, ko2, R], bf16, kind="ExternalInput")
    # bprojl_t is pre-scaled by LORA_SCALING=2.0 on the host
    bprojl_t = nc.dram_tensor("bprojl_t", [R, dmodel_], bf16, kind="ExternalInput")
    biasproj_t = nc.dram_tensor("biasproj_t", [128, dmodel_], f32, kind="ExternalInput")
    # scal columns: 0: s1 = s_x*s_wfc, 1: c1 = 2/s1, 2: s_wproj  (host replicates x128)
    scal_t = nc.dram_tensor("scal_t", [128, 4], f32, kind="ExternalInput")
    ident_t = nc.dram_tensor("ident_t", [128, 128], f32, kind="ExternalInput")
    out_t = nc.dram_tensor("out", [mt, 128, dmodel_], f32, kind="ExternalOutput")

    with tile.TileContext(nc) as tc:
        with ExitStack() as ctx:
            consts = ctx.enter_context(tc.tile_pool(name="consts", bufs=1))
            dram = ctx.enter_context(tc.tile_pool(name="dram", bufs=1, space="DRAM"))

            # whole-kernel residents
            scal_sb = consts.tile([128, 4], f32)
            ident_sb = consts.tile([128, 128], f32)
            maxcol = consts.tile([128, m64], f32)
            afc_sb = consts.tile([128, ko1, R], bf16)
            aproj_sb = consts.tile([128, ko2, R], bf16)
            bprojl_sb = consts.tile([R, dmodel_], bf16)
            biasfc_sb = consts.tile([128, m64], f32)
            biasproj_sb = consts.tile([128, dmodel_], f32)
            xa1bf = consts.tile([R, tc_], bf16)
            xa2bf = consts.tile([R, tc_], bf16)
            hkeep_tiles = [
                consts.tile([128, tc_], f32, tag=f"hk{i}", name=f"hk{i}")
                for i in range(hkeep)
            ]
            h_dram = dram.tile([m64, 128, tc_], f32)
            ar_in = dram.tile([128, 1], f32)
            ar_out = dram.tile(
                [n_cores, 128], f32, addr_space="Shared" if n_cores > 4 else "Local"
            )

            # constants needed early (tiny); the rest stream in later so their
            # kicks don't delay the odd-ko weight stream on the gpsimd queue
            nc.gpsimd.dma_start(scal_sb[:], scal_t[:])
            nc.gpsimd.dma_start(ident_sb[:], ident_t[:])
            nc.gpsimd.dma_start(afc_sb[:], afc_t[:])
            nc.gpsimd.dma_start(biasfc_sb[:], biasfc_t[:])
            # prewarm the Gelu ACT table (otherwise a lazy ~1.3us
            # ACT_TABLE_LOAD stalls the scalar queue at first use mid-L1)
            warm_sb = consts.tile([1, 1], f32)
            nc.scalar.activation(
                warm_sb[:], scal_sb[0:1, 0:1], AF.Gelu, bias=0.0, scale=1.0
            )

            # ---- phase 1: hT = Gelu(s1 * (qx@qW + lora1/s1) + b_fc) ----------
            with tc.tile_pool(name="ph1c", bufs=1) as ph1c, tc.tile_pool(
                name="xtp", bufs=16
            ) as xtp, tc.tile_pool(name="wfc", bufs=24) as wp, tc.tile_pool(
                name="hb1", bufs=6
            ) as hp, tc.tile_pool(name="bfl", bufs=4) as bflp, tc.tile_pool(
                name="h16p", bufs=8
            ) as h16p, tc.tile_pool(
                name="ps1a", bufs=2, space="PSUM"
            ) as ppA, tc.tile_pool(name="ps1b", bufs=1, space="PSUM") as ppB:
                qx_tiles = [
                    ph1c.tile([128, tc_], bf16, tag=f"qx{k}", name=f"qx{k}")
                    for k in range(ko1)
                ]
                ps_a2 = None
                pending_a2 = []  # (mi, h16) deferred one mo to decouple engines

                def flush_a2():
                    nonlocal ps_a2
                    for mi_, h16_ in pending_a2:
                        if ps_a2 is None:
                            ps_a2 = ppB.tile([R, tc_], f32, tag="psa", name="psa2")
                        nc.tensor.matmul(
                            ps_a2[:, :],
                            aproj_sb[:, mi_, :],
                            h16_[:],
                            start=(mi_ == 0),
                            stop=(mi_ == m64 - 1),
                            skip_group_check=True,
                        )
                    pending_a2.clear()

                for mo in range(mo1):
                    flush_a2()
                    pss = [
                        ppA.tile([128, tc_], f32, tag=f"ps1_{i}", name="ps1t")
                        for i in range(3)
                    ]
                    pss.append(ppB.tile([128, tc_], f32, tag="ps1_3", name="ps1t3"))
                    if mo == 8:
                        nc.gpsimd.dma_start(bprojl_sb[:], bprojl_t[:])
                        nc.gpsimd.dma_start(biasproj_sb[:], biasproj_t[:])
                    bfcl_mo = bflp.tile([R, 512], bf16, tag="bfcl", name="bfcl_mo")
                    nc.gpsimd.dma_start(bfcl_mo[:], bfcl_t[:, ds(mo * 512, 512)])
                    xt_tiles = []
                    for ko in range(ko1):
                        if mo == 0:
                            nc.scalar.dma_start(qx_tiles[ko][:], qx_t[:, ko, :])
                        w_ko = wp.tile([128, 512], bf16, tag="wfc", name="w_ko")
                        # weight stream split over two DMA queues
                        (nc.sync if ko % 2 == 0 else nc.gpsimd).dma_start(
                            w_ko[:], wfc_t[mo, ko]
                        )
                        if mo == 0:
                            xt_sb = xtp.tile([128, tc_], bf16, tag="xt", name="xt_sb")
                            nc.sync.dma_start(xt_sb[:], xt_t[:, ko, :])
                            xt_tiles.append(xt_sb)
                        for sub in range(4):
                            nc.tensor.matmul(
                                pss[sub][:],
                                w_ko[:, ts(sub, 128)],
                                qx_tiles[ko][:],
                                start=(ko == 0),
                                stop=False,
                            )
                    if mo == 0:
                        # deferred consts: kicks land behind mo=0's odd-ko
                        # weight stream, ahead of their first readers below
                        nc.gpsimd.dma_start(aproj_sb[:], aproj_t[:])
                        # xa1 = (x @ A_fc)^T * (2/s1), cast to bf16
                        ps_a1 = ppB.tile([R, tc_], f32, tag="psa", name="psa1")
                        for ko in range(ko1):
                            nc.tensor.matmul(
                                ps_a1[:, :],
                                afc_sb[:, ko, :],
                                xt_tiles[ko][:],
                                start=(ko == 0),
                                stop=(ko == ko1 - 1),
                            )
                        nc.vector.tensor_scalar_mul(
                            xa1bf[:], ps_a1[:, :], scal_sb[:R, 1:2]
                        )
                    for sub in range(4):
                        mi = 4 * mo + sub
                        nc.tensor.matmul(
                            pss[sub][:],
                            bfcl_mo[:, ds(sub * 128, 128)],
                            xa1bf[:],
                            start=False,
                            stop=True,
                        )
                        if mi < hkeep:
                            h_sb = hkeep_tiles[mi]
                        else:
                            h_sb = hp.tile([128, tc_], f32, tag="h", name="h_sb")
                        nc.scalar.activation(
                            h_sb[:],
                            pss[sub][:],
                            AF.Gelu if act == "gelu" else AF.Tanh,
                            bias=biasfc_sb[:, mi : mi + 1],
                            scale=scal_sb[:, 0:1],
                        )
                        h16 = h16p.tile([128, tc_], bf16, tag="h16", name="h16")
                        nc.vector.tensor_copy(h16[:], h_sb[:])
                        nc.vector.tensor_reduce(
                            maxcol[:, mi : mi + 1],
                            h_sb[:],
                            axis=mybir.AxisListType.X,
                            op=ALU.max,
                            apply_absolute_value=True,
                        )
                        pending_a2.append((mi, h16))
                        if mi >= hkeep:
                            nc.scalar.dma_start(h_dram[mi], h_sb[:])
                flush_a2()
                # xa2 = (h @ A_proj)^T, cast bf16 (the 2.0 lives in bprojl)
                nc.vector.tensor_copy(xa2bf[:], ps_a2[:, :])

            # ---- phase 1.5 + phase 2 ----------------------------------------
            with tc.tile_pool(name="ph2c", bufs=1) as ph2c, tc.tile_pool(
                name="w2", bufs=16
            ) as w2p, tc.tile_pool(name="hback", bufs=7) as hbp, tc.tile_pool(
                name="qt", bufs=3
            ) as qtp, tc.tile_pool(name="ps2", bufs=2, space="PSUM") as pp2, tc.tile_pool(
                name="ot", bufs=3
            ) as otp:
                # global scale: fold the 128-partition max to a scalar
                # replicated on every partition (exact PE-transpose chain),
                # then a tiny elementwise AllReduce(max) across the 8 cores.
                pmax = ph2c.tile([128, 1], f32)
                nc.vector.tensor_reduce(
                    pmax[:], maxcol[:], axis=mybir.AxisListType.X, op=ALU.max
                )
                # AllGather the raw per-partition maxes ([8,128] core-major),
                # then reduce all 1024 on-core and replicate across partitions
                # via an exact free-broadcast + PE transpose.
                arg_sb = ph2c.tile([1, n_cores * 128], f32)
                if "no_collective" in flags:
                    tp1 = pp2.tile([1, 128], f32, tag="ps2_0", name="tp1")
                    nc.tensor.transpose(tp1[:], pmax[:], ident_sb[:])
                    nc.vector.tensor_copy(arg_sb[0:1, 0:128], tp1[:])
                    nc.vector.tensor_copy(
                        arg_sb[0:1, 128:], arg_sb[0:1, 0:1].to_broadcast(
                            (1, (n_cores - 1) * 128)
                        ),
                    )
                else:
                    nc.gpsimd.dma_start(ar_in[:], pmax[:])
                    nc.gpsimd.collective_compute(
                        "AllGather",
                        ALU.bypass,
                        replica_groups=[list(range(n_cores))],
                        ins=[ar_in[:]],
                        outs=[ar_out[:]],
                    )
                    nc.gpsimd.dma_start(arg_sb[:], ar_out[:])
                m1 = ph2c.tile([1, 1], f32)
                nc.vector.tensor_reduce(
                    m1[:], arg_sb[:], axis=mybir.AxisListType.X, op=ALU.max
                )
                mrow = ph2c.tile([1, 128], f32)
                nc.vector.tensor_copy(mrow[:], m1[:].to_broadcast((1, 128)))
                tp2 = pp2.tile([128, 1], f32, tag="ps2_1", name="tp2")
                nc.tensor.transpose(tp2[:], mrow[:], ident_sb[0:1, 0:1])
                armax = ph2c.tile([128, 1], f32)
                nc.scalar.activation(armax[:], tp2[:], AF.Copy, bias=0.0, scale=1.0)

                # ---- bridge work, independent of the collective result ------
                # lora2b[tok, d] = xa2^T @ (2 B_proj)   (+ b_proj added later)
                l2b = []
                for mi in range(mt):
                    l2b_mi = ph2c.tile(
                        [128, dmodel_], f32, tag=f"l2b{mi}", name=f"l2b{mi}"
                    )
                    for no in range(no2):
                        psL = pp2.tile(
                            [128, 512], f32, tag=f"ps2_{no}", name="psL"
                        )
                        nc.tensor.matmul(
                            psL[:],
                            xa2bf[:, ts(mi, 128)],
                            bprojl_sb[:, ds(no * 512, 512)],
                            start=True,
                            stop=True,
                        )
                        nc.scalar.activation(
                            l2b_mi[:, ds(no * 512, 512)],
                            psL[:],
                            AF.Copy,
                            bias=0.0,
                            scale=1.0,
                        )
                    l2b.append(l2b_mi)

                # scales from the global max
                gmax = ph2c.tile([128, 1], f32)
                scaleh = ph2c.tile([128, 1], f32)
                invsh = ph2c.tile([128, 1], f32)
                s2v = ph2c.tile([128, 1], f32)
                nc.vector.tensor_scalar_max(gmax[:], armax[:], 1e-8)
                # scale_h = gmax / 127  (multiply by fp32(1/127): <=1ulp from divide)
                nc.vector.tensor_scalar_mul(
                    scaleh[:], gmax[:], float(np.float32(1.0) / np.float32(127.0))
                )
                nc.vector.reciprocal(invsh[:], scaleh[:])
                nc.vector.tensor_tensor(
                    s2v[:], scaleh[:], scal_sb[:, 2:3], op=ALU.mult
                )

                # ---- L2 mains: out = s2 * (qh@qW2) + lora2b -----------------
                # qh production (gated on s_h) is fused into the no=0 sweep so
                # the hb-reload and w2 DMA kicks interleave on their queues.
                qh_sb = ph2c.tile([128, ko2, tc_], bf16, tag="qh", name="qh_sb")
                qh_tiles = [qh_sb[:, k, :] for k in range(ko2)]
                for no in range(no2):
                    ps_list = [
                        pp2.tile([128, 512], f32, tag=f"ps2_{mi}", name="ps2t")
                        for mi in range(mt)
                    ]
                    for ko in range(ko2):
                        if no == 0:
                            if ko < hkeep:
                                hsrc = hkeep_tiles[ko]
                            else:
                                hsrc = hbp.tile([128, tc_], f32, tag="hb", name="hb")
                                nc.sync.dma_start(hsrc[:], h_dram[ko])
                            qt = qtp.tile([128, tc_], f32, tag="qt", name="qt")
                            if ko % 2 == 0:
                                # ACT scales, DVE rounds (identical fp32 ops)
                                nc.scalar.activation(
                                    qt[:],
                                    hsrc[:],
                                    AF.Copy,
                                    bias=0.0,
                                    scale=invsh[:, 0:1],
                                )
                                nc.vector.tensor_scalar(
                                    qh_tiles[ko][:],
                                    qt[:],
                                    MAGIC,
                                    MAGIC,
                                    op0=ALU.add,
                                    op1=ALU.subtract,
                                )
                            else:
                                nc.vector.tensor_scalar(
                                    qt[:],
                                    hsrc[:],
                                    invsh[:, 0:1],
                                    MAGIC,
                                    op0=ALU.mult,
                                    op1=ALU.add,
                                )
                                nc.vector.tensor_scalar(
                                    qh_tiles[ko][:],
                                    qt[:],
                                    MAGIC,
                                    None,
                                    op0=ALU.subtract,
                                )
                        w2_sb = w2p.tile([128, 512], bf16, tag="w2", name="w2_sb")
                        (nc.sync if ko % 2 == 0 else nc.gpsimd).dma_start(
                            w2_sb[:], wproj_t[ko, :, no, :]
                        )
                        for mi in range(mt):
                            nc.tensor.matmul(
                                ps_list[mi][:],
                                qh_tiles[ko][:, ts(mi, 128)],
                                w2_sb[:],
                                start=(ko == 0),
                                stop=(ko == ko2 - 1),
                            )
                    if no == 0:
                        for mi in range(mt):
                            nc.vector.tensor_add(
                                l2b[mi][:], l2b[mi][:], biasproj_sb[:]
                            )
                    for mi in range(mt):
                        ot = otp.tile([128, 512], f32, tag="ot", name="ot")
                        nc.scalar.activation(
                            ot[:], ps_list[mi][:], AF.Copy, bias=0.0, scale=s2v[:, 0:1]
                        )
                        nc.vector.tensor_add(
                            ot[:], ot[:], l2b[mi][:, ds(no * 512, 512)]
                        )
                        nc.scalar.dma_start(out_t[mi, :, ds(no * 512, 512)], ot[:])

    nc.compile()
    return nc


def _scale_of(a):
    m = np.max(np.abs(a)).astype(np.float32)
    m = np.maximum(m, np.float32(1e-8))
    return (m / QMAX).astype(np.float32)


def _quant(a, s):
    return np.clip(np.round(a / s), -QMAX, QMAX)


def _prep_inputs(hidden_states, W_fc, b_fc, A_fc, B_fc, W_proj, b_proj, A_proj, B_proj):
    bf16 = ml_dtypes.bfloat16
    x = np.ascontiguousarray(np.asarray(hidden_states, np.float32).reshape(T, D))
    W_fc = np.asarray(W_fc, np.float32)
    W_proj = np.asarray(W_proj, np.float32)

    s_x = _scale_of(x)
    s_wfc = _scale_of(W_fc)
    s_wp = _scale_of(W_proj)
    qx = _quant(x, s_x)  # fp32 integer-valued
    qwfc = _quant(W_fc, s_wfc)
    qwp = _quant(W_proj, s_wp)

    s1 = np.float32(s_x * s_wfc)
    c1 = np.float32(np.float32(2.0) / s1)
    scal_row = np.array([s1, c1, s_wp, 0.0], np.float32)
    scal = np.ascontiguousarray(np.tile(scal_row, (128, 1)))

    wfc_dev = np.ascontiguousarray(
        qwfc.reshape(KO1, 128, MO1, 512).transpose(2, 0, 1, 3).astype(bf16)
    )
    wproj_dev = np.ascontiguousarray(qwp.reshape(KO2, 128, NO2, 512).astype(bf16))
    afc_dev = np.ascontiguousarray(
        np.asarray(A_fc, np.float32).reshape(KO1, 128, R).transpose(1, 0, 2).astype(bf16)
    )
    aproj_dev = np.ascontiguousarray(
        np.asarray(A_proj, np.float32).reshape(KO2, 128, R).transpose(1, 0, 2).astype(bf16)
    )
    bfcl_dev = np.ascontiguousarray(np.asarray(B_fc, np.float32).astype(bf16))
    bprojl_dev = np.ascontiguousarray(
        (np.asarray(B_proj, np.float32) * np.float32(2.0)).astype(bf16)
    )
    biasfc_dev = np.ascontiguousarray(np.asarray(b_fc, np.float32).reshape(M64, 128).T)
    biasproj_dev = np.ascontiguousarray(
        np.tile(np.asarray(b_proj, np.float32)[None, :], (128, 1))
    )
    ident = np.ascontiguousarray(np.eye(128, dtype=np.float32))

    shared = {
        "wfc_t": wfc_dev,
        "afc_t": afc_dev,
        "bfcl_t": bfcl_dev,
        "biasfc_t": biasfc_dev,
        "wproj_t": wproj_dev,
        "aproj_t": aproj_dev,
        "bprojl_t": bprojl_dev,
        "biasproj_t": biasproj_dev,
        "scal_t": scal,
        "ident_t": ident,
    }
    in_maps = []
    for c in range(NCORES):
        xc = x[c * TC : (c + 1) * TC]  # [TC, D]
        qxc = qx[c * TC : (c + 1) * TC]
        qxT = np.ascontiguousarray(
            qxc.T.reshape(KO1, 128, TC).transpose(1, 0, 2).astype(bf16)
        )
        xT = np.ascontiguousarray(
            xc.T.reshape(KO1, 128, TC).transpose(1, 0, 2).astype(bf16)
        )
        in_maps.append({**shared, "qx_t": qxT, "xt_t": xT})
    return in_maps


def _get_runner(**build_kwargs):
    """Build the Bass program once and wrap it in a cached jitted shard_map
    executable (adapted from concourse.bass2jax.run_bass_via_pjrt)."""
    key = ("runner", tuple(sorted(build_kwargs.items())))
    if key in _CACHE:
        return _CACHE[key]

    import jax
    import jax.numpy as jnp  # noqa: F401
    from jax.experimental.shard_map import shard_map
    from jax.sharding import Mesh, PartitionSpec

    from concourse import bass2jax, mybir

    nc = _build_nc(**build_kwargs)
    n_cores_ = build_kwargs.get("n_cores", NCORES)
    bass2jax.install_neuronx_cc_hook()
    assert nc.dbg_addr is None
    partition_name = nc.partition_id_tensor.name if nc.partition_id_tensor else None

    in_names = []
    out_names = []
    out_avals = []
    for alloc in nc.m.functions[0].allocations:
        if not isinstance(alloc, mybir.MemoryLocationSet):
            continue
        name = alloc.memorylocations[0].name
        if alloc.kind == "ExternalInput":
            if name != partition_name:
                in_names.append(name)
        elif alloc.kind == "ExternalOutput":
            out_names.append(name)
            out_avals.append(
                jax.core.ShapedArray(tuple(alloc.tensor_shape), mybir.dt.np(alloc.dtype))
            )
    all_in_names = tuple(in_names) + tuple(out_names)
    if partition_name is not None:
        all_in_names = all_in_names + (partition_name,)
    n_params = len(in_names)
    n_outs = len(out_names)

    def _body(*args):
        operands = list(args)
        if partition_name is not None:
            operands.append(bass2jax.partition_id_tensor())
        outs = bass2jax._bass_exec_p.bind(
            *operands,
            out_avals=tuple(out_avals),
            in_names=all_in_names,
            out_names=tuple(out_names),
            lowering_input_output_aliases=(),
            sim_require_finite=True,
            sim_require_nnan=True,
            nc=nc,
        )
        return tuple(outs)

    devices = jax.devices()[:n_cores_]
    assert len(devices) == n_cores_, f"need {n_cores_} devices, have {len(jax.devices())}"
    mesh = Mesh(np.asarray(devices), ("core",))
    in_specs = (PartitionSpec("core"),) * (n_params + n_outs)
    out_specs = (PartitionSpec("core"),) * n_outs
    donate = tuple(range(n_params, n_params + n_outs))
    fn = jax.jit(
        shard_map(
            _body, mesh=mesh, in_specs=in_specs, out_specs=out_specs, check_rep=False
        ),
        donate_argnums=donate,
        keep_unused=True,
    )
    runner = {
        "fn": fn,
        "nc": nc,
        "in_names": in_names,
        "out_names": out_names,
        "out_avals": out_avals,
        "mesh": mesh,
    }
    runner["n_cores"] = n_cores_
    _CACHE[key] = runner
    return runner


def _zero_outs(runner):
    n = runner["n_cores"]
    return [
        np.zeros((n * a.shape[0], *a.shape[1:]), a.dtype) for a in runner["out_avals"]
    ]


def _concat_inputs(in_maps, in_names):
    return [
        np.concatenate([m[name] for m in in_maps], axis=0) for name in in_names
    ]


def kernel(hidden_states, W_fc, b_fc, A_fc, B_fc, W_proj, b_proj, A_proj, B_proj):
    global LAST_RESULT
    runner = _get_runner()
    in_maps = _prep_inputs(
        hidden_states, W_fc, b_fc, A_fc, B_fc, W_proj, b_proj, A_proj, B_proj
    )
    concat_in = _concat_inputs(in_maps, runner["in_names"])
    out_arrs = runner["fn"](*concat_in, *_zero_outs(runner))
    (out_global,) = [np.asarray(a) for a in out_arrs]
    # out_global: [NCORES*MT, 128, D] -> per-core [MT,128,D] -> tokens x D
    out = out_global.reshape(T, D).astype(np.float32)
    return out.reshape(B_, S, D)


def profile(in_maps=None):
    """Run once under NTFF profiling; returns device exec_time_ns (max across
    profiled cores) or None if the profiling hook is unavailable."""
    import types

    try:
        from trn_agent_boot.trn_boot import _ntff_profile_via_ctypes

        if "antenv.axon_hooks" not in sys.modules:
            import antenv  # noqa: F401

            hooks_mod = types.ModuleType("antenv.axon_hooks")
            hook = _ntff_profile_via_ctypes("/opt/axon/libaxon_pjrt.so")
            hooks_mod.get_axon_ntff_profile_hook = lambda: hook
            sys.modules["antenv.axon_hooks"] = hooks_mod
    except Exception:
        pass

    from concourse import bass_utils

    bass_utils.upload_artifacts = lambda d: d  # no artifact store in container

    if in_maps is None:
        in_maps = _prep_inputs(**_dummy_inputs())
    nc = _build_nc()
    import tempfile

    tmpdir = tempfile.mkdtemp(prefix="ntff_prof_")
    try:
        res = bass_utils.run_bass_kernel_spmd(
            nc, in_maps, list(range(NCORES)), tmpdir=tmpdir, trace=True
        )
        return res.exec_time_ns
    except Exception as e:
        print(f"profile failed: {type(e).__name__}: {e}", file=sys.stderr)
        return None


def _dummy_inputs():
    rng = np.random.default_rng(0)
    return {
        "hidden_states": rng.standard_normal((B_, S, D), dtype=np.float32),
        "W_fc": rng.standard_normal((D, DFF), dtype=np.float32) / 45.0,
        "b_fc": np.zeros(DFF, np.float32),
        "A_fc": rng.standard_normal((D, R), dtype=np.float32) / 45.0,
        "B_fc": rng.standard_normal((R, DFF), dtype=np.float32) * 0.01,
        "W_proj": rng.standard_normal((DFF, D), dtype=np.float32) / 90.0,
        "b_proj": np.zeros(D, np.float32),
        "A_proj": rng.standard_normal((DFF, R), dtype=np.float32) / 90.0,
        "B_proj": rng.standard_normal((R, D), dtype=np.float32) * 0.01,
    }


def bench(n_iters=20, in_maps=None):
    """Steady-state per-iteration wall time of the compiled executable with
    device-resident inputs (upper bound on HW exec time; includes dispatch)."""
    import time

    import jax

    runner = _get_runner()
    if in_maps is None:
        in_maps = _prep_inputs(**_dummy_inputs())
    concat_in = _concat_inputs(in_maps, runner["in_names"])
    from jax.sharding import NamedSharding, PartitionSpec

    sharding = NamedSharding(runner["mesh"], PartitionSpec("core"))
    dev_in = [jax.device_put(a, sharding) for a in concat_in]
    # donated output buffers are consumed per call: pre-stage one set per iter
    zero_sets = [
        [jax.device_put(z, sharding) for z in _zero_outs(runner)]
        for _ in range(n_iters + 1)
    ]
    out = runner["fn"](*dev_in, *zero_sets[-1])
    jax.block_until_ready(out)
    t0 = time.time()
    for i in range(n_iters):
        out = runner["fn"](*dev_in, *zero_sets[i])
    jax.block_until_ready(out)
    dt = (time.time() - t0) / n_iters
    return dt


# revision 18
# speedup vs baseline: 1.0064x; 1.0064x over previous
"""QLoRA-style MLP (fake-quant base + fp32 LoRA + exact GeLU) on 8 TRN2 cores.

Sharding: token data-parallel (4096 tokens / 8 cores = 512 tokens per core),
weights replicated.  The only cross-core communication is a tiny AllReduce(max)
for the global fake-quant scale of the hidden activation.

Math per layer (matching the jax reference):
    base = fq(x) @ fq(W) + b          fq(t) = clip(round(t/s), -127, 127) * s,
                                      s = max(max|t|, 1e-8) / 127  (global max)
    lora = 2.0 * (x @ A) @ B          (low-rank path, bf16 operands on device)
    out  = base + lora                (layer 1 additionally GeLU'd)

Device mapping (per core, T=512 tokens):
  L1: psum[ff128, T] = sum_k qW_fc[k,ff]^T-tiles @ qxT[k,T]   (bf16 exact ints)
                       + B_fc[16,ff]^T @ ((x@A_fc)^T * 2/s1)  (bf16, K=16)
      hT = Gelu(psum * s1 + b_fc); abs-max tracked; per-mi epilogue also
      accumulates xa2 = A_proj^T @ h (bf16) into a persistent psum bank and
      spills h (fp32) to HBM for mi >= HKEEP (first HKEEP tiles stay in SBUF).
  barrier: local max -> PE-transpose/DVE-reduce to a scalar replicated on all
      128 partitions (exact) -> AllReduce(max) over 8 cores -> scales.
      Meanwhile (s_h-independent): lora2b[tok,d] = xa2^T @ (2 B_proj) + b_proj
      is computed into SBUF, and h reloads stream in on the gpsimd queue.
  L2: qhT = round(hT/s_h) as bf16 (magic-number rounding)
      psum[tok128, d] = sum_k qhT[k,tok]-tiles @ qW_proj[k,d]
      out = psum * s2 + lora2b
"""

import os
import sys

import numpy as np

if "/opt/trn_rl_repo" not in sys.path:
    sys.path.insert(0, "/opt/trn_rl_repo")

import ml_dtypes

# Problem shapes (hardcoded per contract).
B_, S, D, DFF, R = 2, 2048, 2048, 8192, 16
T = B_ * S  # 4096 tokens
NCORES = 8
TC = T // NCORES  # 512 tokens per core
QMAX = np.float32(127.0)
MAGIC = float(np.float32(12582912.0))  # 1.5 * 2**23: fp32 round-half-even trick

KO1 = D // 128  # 16  k-tiles for layer 1
MO1 = DFF // 512  # 16  512-wide ff blocks
M64 = DFF // 128  # 64  128-wide ff blocks
KO2 = DFF // 128  # 64  k-tiles for layer 2
NO2 = D // 512  # 4   512-wide output-col blocks
MT = TC // 128  # 4   token tiles per core
HKEEP = 20  # leading h tiles kept resident in SBUF (not spilled)

_CACHE = {}
LAST_RESULT = None  # test harness can read exec_time_ns etc. from here


def _build_nc(n_cores=NCORES, tc_=TC, d_=D, dff_=DFF, dmodel_=D, act="gelu", flags=()):
    """Build + compile the Bass program. Dimensions parameterizable for sim tests."""
    from contextlib import ExitStack

    import concourse.bass as bass  # noqa: F401
    import concourse.mybir as mybir
    import concourse.tile as tile
    from concourse import bacc
    from concourse.bass import ds, ts

    f32 = mybir.dt.float32
    bf16 = mybir.dt.bfloat16
    AF = mybir.ActivationFunctionType
    ALU = mybir.AluOpType

    ko1 = d_ // 128
    mo1 = dff_ // 512
    m64 = dff_ // 128
    ko2 = dff_ // 128
    no2 = dmodel_ // 512
    mt = tc_ // 128
    hkeep = min(HKEEP, m64)

    nc = bacc.Bacc(None, target_bir_lowering=False, num_devices=n_cores)

    # ---- kernel I/O -------------------------------------------------------
    qx_t = nc.dram_tensor("qx_t", [128, ko1, tc_], bf16, kind="ExternalInput")
    xt_t = nc.dram_tensor("xt_t", [128, ko1, tc_], bf16, kind="ExternalInput")
    wfc_t = nc.dram_tensor("wfc_t", [mo1, ko1, 128, 512], bf16, kind="ExternalInput")
    afc_t = nc.dram_tensor("afc_t", [128, ko1, R], bf16, kind="ExternalInput")
    bfcl_t = nc.dram_tensor("bfcl_t", [R, dff_], bf16, kind="ExternalInput")
    biasfc_t = nc.dram_tensor("biasfc_t", [128, m64], f32, kind="ExternalInput")
    wproj_t = nc.dram_tensor("wproj_t", [ko2, 128, no2, 512], bf16, kind="ExternalInput")
    aproj_t = nc.dram_tensor("aproj_t", [128, ko2, R], bf16, kind="ExternalInput")
    # bprojl_t is pre-scaled by LORA_SCALING=2.0 on the host
    bprojl_t = nc.dram_tensor("bprojl_t", [R, dmodel_], bf16, kind="ExternalInput")
    biasproj_t = nc.dram_tensor("biasproj_t", [128, dmodel_], f32, kind="ExternalInput")
    # scal columns: 0: s1 = s_x*s_wfc, 1: c1 = 2/s1, 2: s_wproj  (host replicates x128)
    scal_t = nc.dram_tensor("scal_t", [128, 4], f32, kind="ExternalInput")
    ident_t = nc.dram_tensor("ident_t", [128, 128], f32, kind="ExternalInput")
    out_t = nc.dram_tensor("out", [mt, 128, dmodel_], f32, kind="ExternalOutput")

    with tile.TileContext(nc) as tc:
        with ExitStack() as ctx:
            consts = ctx.enter_context(tc.tile_pool(name="consts", bufs=1))
            dram = ctx.enter_context(tc.tile_pool(name="dram", bufs=1, space="DRAM"))

            # whole-kernel residents
            scal_sb = consts.tile([128, 4], f32)
            ident_sb = consts.tile([128, 128], f32)
            maxcol = consts.tile([128, m64], f32)
            afc_sb = consts.tile([128, ko1, R], bf16)
            aproj_sb = consts.tile([128, ko2, R], bf16)
            bprojl_sb = consts.tile([R, dmodel_], bf16)
            biasfc_sb = consts.tile([128, m64], f32)
            biasproj_sb = consts.tile([128, dmodel_], f32)
            xa1bf = consts.tile([R, tc_], bf16)
            xa2bf = consts.tile([R, tc_], bf16)
            hkeep_tiles = [
                consts.tile([128, tc_], f32, tag=f"hk{i}", name=f"hk{i}")
                for i in range(hkeep)
            ]
            h_dram = dram.tile([m64, 128, tc_], f32)
            ar_in = dram.tile([128, 1], f32)
            ar_out = dram.tile(
                [n_cores, 128], f32, addr_space="Shared" if n_cores > 4 else "Local"
            )

            # constants needed early (tiny); the rest stream in later so their
            # kicks don't delay the odd-ko weight stream on the gpsimd queue
            nc.gpsimd.dma_start(scal_sb[:], scal_t[:])
            nc.gpsimd.dma_start(ident_sb[:], ident_t[:])
            nc.gpsimd.dma_start(afc_sb[:], afc_t[:])
            nc.gpsimd.dma_start(biasfc_sb[:], biasfc_t[:])
            # prewarm the Gelu ACT table (otherwise a lazy ~1.3us
            # ACT_TABLE_LOAD stalls the scalar queue at first use mid-L1)
            warm_sb = consts.tile([1, 1], f32)
            nc.scalar.activation(
                warm_sb[:], scal_sb[0:1, 0:1], AF.Gelu, bias=0.0, scale=1.0
            )

            # ---- phase 1: hT = Gelu(s1 * (qx@qW + lora1/s1) + b_fc) ----------
            with tc.tile_pool(name="ph1c", bufs=1) as ph1c, tc.tile_pool(
                name="xtp", bufs=1
            ) as xtp, tc.tile_pool(name="wfc", bufs=24) as wp, tc.tile_pool(
                name="hb1", bufs=6
            ) as hp, tc.tile_pool(name="bfl", bufs=4) as bflp, tc.tile_pool(
                name="h16p", bufs=8
            ) as h16p, tc.tile_pool(
                name="ps1a", bufs=2, space="PSUM"
            ) as ppA, tc.tile_pool(name="ps1b", bufs=1, space="PSUM") as ppB:
                qx_sb = ph1c.tile([128, ko1, tc_], bf16)
                qx_tiles = [qx_sb[:, k, :] for k in range(ko1)]
                xt_sb = xtp.tile([128, ko1, tc_], bf16)
                ps_a2 = None
                pending_a2 = []  # (mi, h16) deferred one mo to decouple engines

                def flush_a2():
                    nonlocal ps_a2
                    for mi_, h16_ in pending_a2:
                        if ps_a2 is None:
                            ps_a2 = ppB.tile([R, tc_], f32, tag="psa", name="psa2")
                        nc.tensor.matmul(
                            ps_a2[:, :],
                            aproj_sb[:, mi_, :],
                            h16_[:],
                            start=(mi_ == 0),
                            stop=(mi_ == m64 - 1),
                            skip_group_check=True,
                        )
                    pending_a2.clear()

                for mo in range(mo1):
                    flush_a2()
                    pss = [
                        ppA.tile([128, tc_], f32, tag=f"ps1_{i}", name="ps1t")
                        for i in range(3)
                    ]
                    pss.append(ppB.tile([128, tc_], f32, tag="ps1_3", name="ps1t3"))
                    if mo == 8:
                        nc.gpsimd.dma_start(bprojl_sb[:], bprojl_t[:])
                        nc.gpsimd.dma_start(biasproj_sb[:], biasproj_t[:])
                    bfcl_mo = bflp.tile([R, 512], bf16, tag="bfcl", name="bfcl_mo")
                    nc.gpsimd.dma_start(bfcl_mo[:], bfcl_t[:, ds(mo * 512, 512)])
                    for ko in range(ko1):
                        if mo == 0 and ko < 4:
                            nc.scalar.dma_start(qx_tiles[ko][:], qx_t[:, ko, :])
                        elif mo == 0 and ko == 4:
                            # bulk-load the rest: fewer kicks avoids a convoy
                            # on the shared 8-deep DMA completion-sem ring
                            nc.scalar.dma_start(qx_sb[:, 4:, :], qx_t[:, 4:, :])
                            nc.scalar.dma_start(xt_sb[:], xt_t[:])
                        w_ko = wp.tile([128, 512], bf16, tag="wfc", name="w_ko")
                        # weight stream split over two DMA queues
                        (nc.sync if ko % 2 == 0 else nc.gpsimd).dma_start(
                            w_ko[:], wfc_t[mo, ko]
                        )

                        for sub in range(4):
                            nc.tensor.matmul(
                                pss[sub][:],
                                w_ko[:, ts(sub, 128)],
                                qx_tiles[ko][:],
                                start=(ko == 0),
                                stop=False,
                            )
                    if mo == 0:
                        # deferred consts: kicks land behind mo=0's odd-ko
                        # weight stream, ahead of their first readers below
                        nc.gpsimd.dma_start(aproj_sb[:], aproj_t[:])
                        # xa1 = (x @ A_fc)^T * (2/s1), cast to bf16
                        ps_a1 = ppB.tile([R, tc_], f32, tag="psa", name="psa1")
                        for ko in range(ko1):
                            nc.tensor.matmul(
                                ps_a1[:, :],
                                afc_sb[:, ko, :],
                                xt_sb[:, ko, :],
                                start=(ko == 0),
                                stop=(ko == ko1 - 1),
                            )
                        nc.vector.tensor_scalar_mul(
                            xa1bf[:], ps_a1[:, :], scal_sb[:R, 1:2]
                        )
                    for sub in range(4):
                        mi = 4 * mo + sub
                        nc.tensor.matmul(
                            pss[sub][:],
                            bfcl_mo[:, ds(sub * 128, 128)],
                            xa1bf[:],
                            start=False,
                            stop=True,
                        )
                        if mi < hkeep:
                            h_sb = hkeep_tiles[mi]
                        else:
                            h_sb = hp.tile([128, tc_], f32, tag="h", name="h_sb")
                        nc.scalar.activation(
                            h_sb[:],
                            pss[sub][:],
                            AF.Gelu if act == "gelu" else AF.Tanh,
                            bias=biasfc_sb[:, mi : mi + 1],
                            scale=scal_sb[:, 0:1],
                        )
                        h16 = h16p.tile([128, tc_], bf16, tag="h16", name="h16")
                        nc.vector.tensor_copy(h16[:], h_sb[:])
                        nc.vector.tensor_reduce(
                            maxcol[:, mi : mi + 1],
                            h_sb[:],
                            axis=mybir.AxisListType.X,
                            op=ALU.max,
                            apply_absolute_value=True,
                        )
                        pending_a2.append((mi, h16))
                        if mi >= hkeep:
                            nc.scalar.dma_start(h_dram[mi], h_sb[:])
                flush_a2()
                # xa2 = (h @ A_proj)^T, cast bf16 (the 2.0 lives in bprojl)
                nc.vector.tensor_copy(xa2bf[:], ps_a2[:, :])

            # ---- phase 1.5 + phase 2 ----------------------------------------
            with tc.tile_pool(name="ph2c", bufs=1) as ph2c, tc.tile_pool(
                name="w2", bufs=16
            ) as w2p, tc.tile_pool(name="hback", bufs=7) as hbp, tc.tile_pool(
                name="qt", bufs=3
            ) as qtp, tc.tile_pool(name="ps2", bufs=2, space="PSUM") as pp2, tc.tile_pool(
                name="ot", bufs=3
            ) as otp:
                # global scale: fold the 128-partition max to a scalar
                # replicated on every partition (exact PE-transpose chain),
                # then a tiny elementwise AllReduce(max) across the 8 cores.
                pmax = ph2c.tile([128, 1], f32)
                nc.vector.tensor_reduce(
                    pmax[:], maxcol[:], axis=mybir.AxisListType.X, op=ALU.max
                )
                # AllGather the raw per-partition maxes ([8,128] core-major),
                # then reduce all 1024 on-core and replicate across partitions
                # via an exact free-broadcast + PE transpose.
                arg_sb = ph2c.tile([1, n_cores * 128], f32)
                if "no_collective" in flags:
                    tp1 = pp2.tile([1, 128], f32, tag="ps2_0", name="tp1")
                    nc.tensor.transpose(tp1[:], pmax[:], ident_sb[:])
                    nc.vector.tensor_copy(arg_sb[0:1, 0:128], tp1[:])
                    nc.vector.tensor_copy(
                        arg_sb[0:1, 128:], arg_sb[0:1, 0:1].to_broadcast(
                            (1, (n_cores - 1) * 128)
                        ),
                    )
                else:
                    nc.gpsimd.dma_start(ar_in[:], pmax[:])
                    nc.gpsimd.collective_compute(
                        "AllGather",
                        ALU.bypass,
                        replica_groups=[list(range(n_cores))],
                        ins=[ar_in[:]],
                        outs=[ar_out[:]],
                    )
                    nc.gpsimd.dma_start(arg_sb[:], ar_out[:])
                m1 = ph2c.tile([1, 1], f32)
                nc.vector.tensor_reduce(
                    m1[:], arg_sb[:], axis=mybir.AxisListType.X, op=ALU.max
                )
                mrow = ph2c.tile([1, 128], f32)
                nc.vector.tensor_copy(mrow[:], m1[:].to_broadcast((1, 128)))
                tp2 = pp2.tile([128, 1], f32, tag="ps2_1", name="tp2")
                nc.tensor.transpose(tp2[:], mrow[:], ident_sb[0:1, 0:1])
                armax = ph2c.tile([128, 1], f32)
                nc.scalar.activation(armax[:], tp2[:], AF.Copy, bias=0.0, scale=1.0)

                # ---- bridge work, independent of the collective result ------
                # lora2b[tok, d] = xa2^T @ (2 B_proj)   (+ b_proj added later)
                l2b = []
                for mi in range(mt):
                    l2b_mi = ph2c.tile(
                        [128, dmodel_], f32, tag=f"l2b{mi}", name=f"l2b{mi}"
                    )
                    for no in range(no2):
                        psL = pp2.tile(
                            [128, 512], f32, tag=f"ps2_{no}", name="psL"
                        )
                        nc.tensor.matmul(
                            psL[:],
                            xa2bf[:, ts(mi, 128)],
                            bprojl_sb[:, ds(no * 512, 512)],
                            start=True,
                            stop=True,
                        )
                        nc.scalar.activation(
                            l2b_mi[:, ds(no * 512, 512)],
                            psL[:],
                            AF.Copy,
                            bias=0.0,
                            scale=1.0,
                        )
                    l2b.append(l2b_mi)

                # scales from the global max
                gmax = ph2c.tile([128, 1], f32)
                scaleh = ph2c.tile([128, 1], f32)
                invsh = ph2c.tile([128, 1], f32)
                s2v = ph2c.tile([128, 1], f32)
                nc.vector.tensor_scalar_max(gmax[:], armax[:], 1e-8)
                # scale_h = gmax / 127  (multiply by fp32(1/127): <=1ulp from divide)
                nc.vector.tensor_scalar_mul(
                    scaleh[:], gmax[:], float(np.float32(1.0) / np.float32(127.0))
                )
                nc.vector.reciprocal(invsh[:], scaleh[:])
                nc.vector.tensor_tensor(
                    s2v[:], scaleh[:], scal_sb[:, 2:3], op=ALU.mult
                )

                # ---- L2 mains: out = s2 * (qh@qW2) + lora2b -----------------
                # qh production (gated on s_h) is fused into the no=0 sweep so
                # the hb-reload and w2 DMA kicks interleave on their queues.
                qh_sb = ph2c.tile([128, ko2, tc_], bf16, tag="qh", name="qh_sb")
                qh_tiles = [qh_sb[:, k, :] for k in range(ko2)]
                for no in range(no2):
                    ps_list = [
                        pp2.tile([128, 512], f32, tag=f"ps2_{mi}", name="ps2t")
                        for mi in range(mt)
                    ]
                    for ko in range(ko2):
                        if no == 0:
                            if ko < hkeep:
                                hsrc = hkeep_tiles[ko]
                            else:
                                hsrc = hbp.tile([128, tc_], f32, tag="hb", name="hb")
                                nc.sync.dma_start(hsrc[:], h_dram[ko])
                            qt = qtp.tile([128, tc_], f32, tag="qt", name="qt")
                            if ko % 2 == 0:
                                # ACT scales, DVE rounds (identical fp32 ops)
                                nc.scalar.activation(
                                    qt[:],
                                    hsrc[:],
                                    AF.Copy,
                                    bias=0.0,
                                    scale=invsh[:, 0:1],
                                )
                                nc.vector.tensor_scalar(
                                    qh_tiles[ko][:],
                                    qt[:],
                                    MAGIC,
                                    MAGIC,
                                    op0=ALU.add,
                                    op1=ALU.subtract,
                                )
                            else:
                                nc.vector.tensor_scalar(
                                    qt[:],
                                    hsrc[:],
                                    invsh[:, 0:1],
                                    MAGIC,
                                    op0=ALU.mult,
                                    op1=ALU.add,
                                )
                                nc.vector.tensor_scalar(
                                    qh_tiles[ko][:],
                                    qt[:],
                                    MAGIC,
                                    None,
                                    op0=ALU.subtract,
                                )
                        w2_sb = w2p.tile([128, 512], bf16, tag="w2", name="w2_sb")
                        (nc.sync if ko % 2 == 0 else nc.gpsimd).dma_start(
                            w2_sb[:], wproj_t[ko, :, no, :]
                        )
                        for mi in range(mt):
                            nc.tensor.matmul(
                                ps_list[mi][:],
                                qh_tiles[ko][:, ts(mi, 128)],
                                w2_sb[:],
                                start=(ko == 0),
                                stop=(ko == ko2 - 1),
                            )
                    if no == 0:
                        for mi in range(mt):
                            nc.vector.tensor_add(
                                l2b[mi][:], l2b[mi][:], biasproj_sb[:]
                            )
                    for mi in range(mt):
                        ot = otp.tile([128, 512], f32, tag="ot", name="ot")
                        nc.scalar.activation(
                            ot[:], ps_list[mi][:], AF.Copy, bias=0.0, scale=s2v[:, 0:1]
                        )
                        nc.vector.tensor_add(
                            ot[:], ot[:], l2b[mi][:, ds(no * 512, 512)]
                        )
                        nc.scalar.dma_start(out_t[mi, :, ds(no * 512, 512)], ot[:])

    nc.compile()
    return nc


def _scale_of(a):
    m = np.max(np.abs(a)).astype(np.float32)
    m = np.maximum(m, np.float32(1e-8))
    return (m / QMAX).astype(np.float32)


def _quant(a, s):
    return np.clip(np.round(a / s), -QMAX, QMAX)


def _prep_inputs(hidden_states, W_fc, b_fc, A_fc, B_fc, W_proj, b_proj, A_proj, B_proj):
    bf16 = ml_dtypes.bfloat16
    x = np.ascontiguousarray(np.asarray(hidden_states, np.float32).reshape(T, D))
    W_fc = np.asarray(W_fc, np.float32)
    W_proj = np.asarray(W_proj, np.float32)

    s_x = _scale_of(x)
    s_wfc = _scale_of(W_fc)
    s_wp = _scale_of(W_proj)
    qx = _quant(x, s_x)  # fp32 integer-valued
    qwfc = _quant(W_fc, s_wfc)
    qwp = _quant(W_proj, s_wp)

    s1 = np.float32(s_x * s_wfc)
    c1 = np.float32(np.float32(2.0) / s1)
    scal_row = np.array([s1, c1, s_wp, 0.0], np.float32)
    scal = np.ascontiguousarray(np.tile(scal_row, (128, 1)))

    wfc_dev = np.ascontiguousarray(
        qwfc.reshape(KO1, 128, MO1, 512).transpose(2, 0, 1, 3).astype(bf16)
    )
    wproj_dev = np.ascontiguousarray(qwp.reshape(KO2, 128, NO2, 512).astype(bf16))
    afc_dev = np.ascontiguousarray(
        np.asarray(A_fc, np.float32).reshape(KO1, 128, R).transpose(1, 0, 2).astype(bf16)
    )
    aproj_dev = np.ascontiguousarray(
        np.asarray(A_proj, np.float32).reshape(KO2, 128, R).transpose(1, 0, 2).astype(bf16)
    )
    bfcl_dev = np.ascontiguousarray(np.asarray(B_fc, np.float32).astype(bf16))
    bprojl_dev = np.ascontiguousarray(
        (np.asarray(B_proj, np.float32) * np.float32(2.0)).astype(bf16)
    )
    biasfc_dev = np.ascontiguousarray(np.asarray(b_fc, np.float32).reshape(M64, 128).T)
    biasproj_dev = np.ascontiguousarray(
        np.tile(np.asarray(b_proj, np.float32)[None, :], (128, 1))
    )
    ident = np.ascontiguousarray(np.eye(128, dtype=np.float32))

    shared = {
        "wfc_t": wfc_dev,
        "afc_t": afc_dev,
        "bfcl_t": bfcl_dev,
        "biasfc_t": biasfc_dev,
        "wproj_t": wproj_dev,
        "aproj_t": aproj_dev,
        "bprojl_t": bprojl_dev,
        "biasproj_t": biasproj_dev,
        "scal_t": scal,
        "ident_t": ident,
    }
    in_maps = []
    for c in range(NCORES):
        xc = x[c * TC : (c + 1) * TC]  # [TC, D]
        qxc = qx[c * TC : (c + 1) * TC]
        qxT = np.ascontiguousarray(
            qxc.T.reshape(KO1, 128, TC).transpose(1, 0, 2).astype(bf16)
        )
        xT = np.ascontiguousarray(
            xc.T.reshape(KO1, 128, TC).transpose(1, 0, 2).astype(bf16)
        )
        in_maps.append({**shared, "qx_t": qxT, "xt_t": xT})
    return in_maps


def _get_runner(**build_kwargs):
    """Build the Bass program once and wrap it in a cached jitted shard_map
    executable (adapted from concourse.bass2jax.run_bass_via_pjrt)."""
    key = ("runner", tuple(sorted(build_kwargs.items())))
    if key in _CACHE:
        return _CACHE[key]

    import jax
    import jax.numpy as jnp  # noqa: F401
    from jax.experimental.shard_map import shard_map
    from jax.sharding import Mesh, PartitionSpec

    from concourse import bass2jax, mybir

    nc = _build_nc(**build_kwargs)
    n_cores_ = build_kwargs.get("n_cores", NCORES)
    bass2jax.install_neuronx_cc_hook()
    assert nc.dbg_addr is None
    partition_name = nc.partition_id_tensor.name if nc.partition_id_tensor else None

    in_names = []
    out_names = []
    out_avals = []
    for alloc in nc.m.functions[0].allocations:
        if not isinstance(alloc, mybir.MemoryLocationSet):
            continue
        name = alloc.memorylocations[0].name
        if alloc.kind == "ExternalInput":
            if name != partition_name:
                in_names.append(name)
        elif alloc.kind == "ExternalOutput":
            out_names.append(name)
            out_avals.append(
                jax.core.ShapedArray(tuple(alloc.tensor_shape), mybir.dt.np(alloc.dtype))
            )
    all_in_names = tuple(in_names) + tuple(out_names)
    if partition_name is not None:
        all_in_names = all_in_names + (partition_name,)
    n_params = len(in_names)
    n_outs = len(out_names)

    def _body(*args):
        operands = list(args)
        if partition_name is not None:
            operands.append(bass2jax.partition_id_tensor())
        outs = bass2jax._bass_exec_p.bind(
            *operands,
            out_avals=tuple(out_avals),
            in_names=all_in_names,
            out_names=tuple(out_names),
            lowering_input_output_aliases=(),
            sim_require_finite=True,
            sim_require_nnan=True,
            nc=nc,
        )
        return tuple(outs)

    devices = jax.devices()[:n_cores_]
    assert len(devices) == n_cores_, f"need {n_cores_} devices, have {len(jax.devices())}"
    mesh = Mesh(np.asarray(devices), ("core",))
    in_specs = (PartitionSpec("core"),) * (n_params + n_outs)
    out_specs = (PartitionSpec("core"),) * n_outs
    donate = tuple(range(n_params, n_params + n_outs))
    fn = jax.jit(
        shard_map(
            _body, mesh=mesh, in_specs=in_specs, out_specs=out_specs, check_rep=False
        ),
        donate_argnums=donate,
        keep_unused=True,
    )
    runner = {
        "fn": fn,
        "nc": nc,
        "in_names": in_names,
        "out_names": out_names,
        "out_avals": out_avals,
        "mesh": mesh,
    }
    runner["n_cores"] = n_cores_
    _CACHE[key] = runner
    return runner


def _zero_outs(runner):
    n = runner["n_cores"]
    return [
        np.zeros((n * a.shape[0], *a.shape[1:]), a.dtype) for a in runner["out_avals"]
    ]


def _concat_inputs(in_maps, in_names):
    return [
        np.concatenate([m[name] for m in in_maps], axis=0) for name in in_names
    ]


def kernel(hidden_states, W_fc, b_fc, A_fc, B_fc, W_proj, b_proj, A_proj, B_proj):
    global LAST_RESULT
    runner = _get_runner()
    in_maps = _prep_inputs(
        hidden_states, W_fc, b_fc, A_fc, B_fc, W_proj, b_proj, A_proj, B_proj
    )
    concat_in = _concat_inputs(in_maps, runner["in_names"])
    out_arrs = runner["fn"](*concat_in, *_zero_outs(runner))
    (out_global,) = [np.asarray(a) for a in out_arrs]
    # out_global: [NCORES*MT, 128, D] -> per-core [MT,128,D] -> tokens x D
    out = out_global.reshape(T, D).astype(np.float32)
    return out.reshape(B_, S, D)


def profile(in_maps=None):
    """Run once under NTFF profiling; returns device exec_time_ns (max across
    profiled cores) or None if the profiling hook is unavailable."""
    import types

    try:
        from trn_agent_boot.trn_boot import _ntff_profile_via_ctypes

        if "antenv.axon_hooks" not in sys.modules:
            import antenv  # noqa: F401

            hooks_mod = types.ModuleType("antenv.axon_hooks")
            hook = _ntff_profile_via_ctypes("/opt/axon/libaxon_pjrt.so")
            hooks_mod.get_axon_ntff_profile_hook = lambda: hook
            sys.modules["antenv.axon_hooks"] = hooks_mod
    except Exception:
        pass

    from concourse import bass_utils

    bass_utils.upload_artifacts = lambda d: d  # no artifact store in container

    if in_maps is None:
        in_maps = _prep_inputs(**_dummy_inputs())
    nc = _build_nc()
    import tempfile

    tmpdir = tempfile.mkdtemp(prefix="ntff_prof_")
    try:
        res = bass_utils.run_bass_kernel_spmd(
            nc, in_maps, list(range(NCORES)), tmpdir=tmpdir, trace=True
        )
        return res.exec_time_ns
    except Exception as e:
        print(f"profile failed: {type(e).__name__}: {e}", file=sys.stderr)
        return None


def _dummy_inputs():
    rng = np.random.default_rng(0)
    return {
        "hidden_states": rng.standard_normal((B_, S, D), dtype=np.float32),
        "W_fc": rng.standard_normal((D, DFF), dtype=np.float32) / 45.0,
        "b_fc": np.zeros(DFF, np.float32),
        "A_fc": rng.standard_normal((D, R), dtype=np.float32) / 45.0,
        "B_fc": rng.standard_normal((R, DFF), dtype=np.float32) * 0.01,
        "W_proj": rng.standard_normal((DFF, D), dtype=np.float32) / 90.0,
        "b_proj": np.zeros(D, np.float32),
        "A_proj": rng.standard_normal((DFF, R), dtype=np.float32) / 90.0,
        "B_proj": rng.standard_normal((R, D), dtype=np.float32) * 0.01,
    }


def bench(n_iters=20, in_maps=None):
    """Steady-state per-iteration wall time of the compiled executable with
    device-resident inputs (upper bound on HW exec time; includes dispatch)."""
    import time

    import jax

    runner = _get_runner()
    if in_maps is None:
        in_maps = _prep_inputs(**_dummy_inputs())
    concat_in = _concat_inputs(in_maps, runner["in_names"])
    from jax.sharding import NamedSharding, PartitionSpec

    sharding = NamedSharding(runner["mesh"], PartitionSpec("core"))
    dev_in = [jax.device_put(a, sharding) for a in concat_in]
    # donated output buffers are consumed per call: pre-stage one set per iter
    zero_sets = [
        [jax.device_put(z, sharding) for z in _zero_outs(runner)]
        for _ in range(n_iters + 1)
    ]
    out = runner["fn"](*dev_in, *zero_sets[-1])
    jax.block_until_ready(out)
    t0 = time.time()
    for i in range(n_iters):
        out = runner["fn"](*dev_in, *zero_sets[i])
    jax.block_until_ready(out)
    dt = (time.time() - t0) / n_iters
    return dt
